# revision 3
# baseline (speedup 1.0000x reference)
"""GCN layer on 8 Trainium2 NeuronCores (Bass/Tile SPMD kernel).

Math: support = scatter_add(features[edge_src] * edge_w, edge_dst);
      out = support @ weight.T
Shapes: features [50000, 64] f32, edge_src/dst [800000] int, edge_w [800000]
f32, weight [64, 64] f32 -> out [50000, 64] f32.

Sharding: nodes (rows) are partitioned across the 8 cores: core c owns dst
rows [c*6250, (c+1)*6250). Features are replicated to every core (12.8 MB,
gather source in HBM), each core receives exactly the edges whose dst it
owns. No collectives are needed; per-core outputs concatenate on host.

Device pipeline per core (one shared SPMD program):
  1. gpsimd.dma_gather: 256B feature rows HBM->SBUF by edge_src.
     Indices are int16, so edges are host-sorted by src and split at row
     32768 into two windows; the gather base AP supplies the window offset.
  2. DVE multiply by edge_w (broadcast over the 64 features).
  3. gpsimd.dma_scatter_add: CCE accumulate into per-core support rows in
     HBM (dst-local indices < 6250 fit int16). Two accumulator buffers are
     alternated so consecutive scatters pipeline (Tile serializes WAW on a
     single tensor).
  4. Final: support tiles -> PE transpose -> matmul with W^T -> out.

All per-core variation (edge counts, index values) flows through input
tensors; the program itself is identical across cores (padded with
zero-weight edges to the max per-core count).
"""

import os
import numpy as np

N_NODES = 50000
D = 64
N_CORES = 8
NLOC = N_NODES // N_CORES  # 6250 dst rows per core
WIN = 32768  # int16 index window for the gather source
CH = 8192  # edges per gather/scatter chunk

_NC_CACHE = {}
LAST_RESULT = None  # BassKernelResults of the most recent run (for test.py)


def _build_nc(ka, kb, n_nodes=N_NODES, nloc=NLOC, win=WIN, ch=CH):
    """Build + compile the shared SPMD program for padded group sizes ka/kb."""
    from contextlib import ExitStack

    import concourse.bass as bass
    import concourse.tile as tile
    from concourse import bacc, masks, mybir

    nw = ka + kb
    f32 = mybir.dt.float32
    i16 = mybir.dt.int16

    nc = bacc.Bacc("TRN2", target_bir_lowering=False, debug=False)

    feat = nc.dram_tensor("feat", [n_nodes, D], f32, kind="ExternalInput")
    gidx = nc.dram_tensor("gidx", [128, nw // 16], i16, kind="ExternalInput")
    sidx = nc.dram_tensor("sidx", [128, nw // 16], i16, kind="ExternalInput")
    ew = nc.dram_tensor("ew", [128, nw // 128], f32, kind="ExternalInput")
    wt = nc.dram_tensor("wt", [D, D], f32, kind="ExternalInput")
    out = nc.dram_tensor("out", [nloc, D], f32, kind="ExternalOutput")
    # ExternalOutputs are pre-zeroed by the runner; used as scatter accumulators.
    sup_a = nc.dram_tensor("sup_a", [nloc, D], f32, kind="ExternalOutput")
    sup_b = nc.dram_tensor("sup_b", [nloc, D], f32, kind="ExternalOutput")

    with tile.TileContext(nc) as tc, ExitStack() as ctx:
        cpool = ctx.enter_context(tc.tile_pool(name="const", bufs=1))
        gpool = ctx.enter_context(tc.tile_pool(name="gather", bufs=3))
        fpool = ctx.enter_context(tc.tile_pool(name="fin", bufs=3))
        ppool = ctx.enter_context(tc.tile_pool(name="psum", bufs=4, space="PSUM"))

        gidx_sb = cpool.tile([128, nw // 16], i16)
        nc.sync.dma_start(gidx_sb[:], gidx[:])
        sidx_sb = cpool.tile([128, nw // 16], i16)
        nc.sync.dma_start(sidx_sb[:], sidx[:])
        ew_sb = cpool.tile([128, nw // 128], f32)
        nc.sync.dma_start(ew_sb[:], ew[:])
        wt_sb = cpool.tile([D, D], f32)
        nc.sync.dma_start(wt_sb[:], wt[:])
        ident = cpool.tile([128, 128], f32)
        masks.make_identity(nc, ident[:])

        # (start, size, window base, window rows)
        chunks = []
        for st in range(0, ka, ch):
            chunks.append((st, min(ch, ka - st), 0, win))
        for st in range(ka, nw, ch):
            chunks.append((st, min(ch, nw - st), win, n_nodes - win))

        for k, (st, sz, base, rows) in enumerate(chunks):
            c = sz // 128
            gt = gpool.tile([128, ch // 128, D], f32, tag="gt")
            nc.gpsimd.dma_gather(
                gt[:, :c, :],
                feat[base : base + rows, :],
                gidx_sb[:, st // 16 : (st + sz) // 16],
                sz,
                sz,
                D,
            )
            ewb = ew_sb[:, st // 128 : (st + sz) // 128].broadcast_to([128, c, D])
            nc.vector.tensor_mul(gt[:, :c, :], gt[:, :c, :], ewb)
            target = sup_a if k % 2 == 0 else sup_b
            nc.gpsimd.dma_scatter_add(
                target[:, :],
                gt[:, :c, :],
                sidx_sb[:, st // 16 : (st + sz) // 16],
                sz,
                sz,
                D,
            )

        # out = (sup_a + sup_b) @ W.T, tile by tile
        ntiles = (nloc + 127) // 128
        for t in range(ntiles):
            r = min(128, nloc - t * 128)
            sa = fpool.tile([128, D], f32, tag="sa")
            nc.sync.dma_start(sa[:r, :], sup_a[t * 128 : t * 128 + r, :])
            sb = fpool.tile([128, D], f32, tag="sb")
            nc.sync.dma_start(sb[:r, :], sup_b[t * 128 : t * 128 + r, :])
            nc.vector.tensor_add(sa[:r, :], sa[:r, :], sb[:r, :])
            tp = ppool.tile([D, 128], f32, tag="tp")
            nc.tensor.transpose(tp[:, :r], sa[:r, :], ident[:r, :r])
            tps = fpool.tile([D, 128], f32, tag="tps")
            nc.vector.tensor_copy(tps[:, :r], tp[:, :r])
            om = ppool.tile([128, D], f32, tag="om")
            nc.tensor.matmul(om[:r, :], tps[:, :r], wt_sb[:, :], start=True, stop=True)
            os_ = fpool.tile([128, D], f32, tag="os")
            nc.vector.tensor_copy(os_[:r, :], om[:r, :])
            nc.sync.dma_start(out[t * 128 : t * 128 + r, :], os_[:r, :])

    nc.compile()
    return nc


def _wrap16(x, nw):
    """int16 index layout: idx i -> [i % 16, i // 16], replicated to 128 parts."""
    a = np.asarray(x, dtype=np.int16).reshape(nw // 16, 16).T  # [16, nw/16]
    return np.ascontiguousarray(np.tile(a, (8, 1)))  # [128, nw/16]


def prepare_inputs(features, edge_src, edge_dst, edge_w, weight,
                   n_cores=N_CORES, nloc=NLOC, win=WIN, ch=CH):
    """Host-side sharding: per-core dst-owned edges, src-sorted, window-split,
    padded to common sizes. Returns (ka, kb, in_maps)."""
    features = np.ascontiguousarray(np.asarray(features, dtype=np.float32))
    src = np.asarray(edge_src).astype(np.int64)
    dst = np.asarray(edge_dst).astype(np.int64)
    w = np.asarray(edge_w, dtype=np.float32)
    weight = np.asarray(weight, dtype=np.float32)

    core = dst // nloc
    order = np.lexsort((src, core))  # by core, then src
    src_s, dst_s, w_s = src[order], dst[order], w[order]
    core_s = core[order]
    bounds = np.searchsorted(core_s, np.arange(n_cores + 1))

    per_core = []
    for c in range(n_cores):
        lo, hi = bounds[c], bounds[c + 1]
        s, d, ww = src_s[lo:hi], dst_s[lo:hi] - c * nloc, w_s[lo:hi]
        na = int(np.searchsorted(s, win))
        per_core.append((s, d, ww, na))

    rup = lambda x, m: ((x + m - 1) // m * m) if x else m
    ka = rup(max(na for (_, _, _, na) in per_core), 128)
    kb = rup(max(len(s) - na for (s, _, _, na) in per_core), 128)
    nw = ka + kb

    wt = np.ascontiguousarray(weight.T)  # [d_in, d_out]
    in_maps = []
    for c in range(n_cores):
        s, d, ww, na = per_core[c]
        nb = len(s) - na
        gsrc = np.zeros(nw, dtype=np.int64)
        sdst = np.zeros(nw, dtype=np.int64)
        wpad = np.zeros(nw, dtype=np.float32)
        gsrc[:na] = s[:na]
        gsrc[ka : ka + nb] = s[na:] - win
        sdst[:na] = d[:na]
        sdst[ka : ka + nb] = d[na:]
        wpad[:na] = ww[:na]
        wpad[ka : ka + nb] = ww[na:]
        in_maps.append(
            {
                "feat": features,
                "gidx": _wrap16(gsrc, nw),
                "sidx": _wrap16(sdst, nw),
                "ew": np.ascontiguousarray(wpad.reshape(nw // 128, 128).T),
                "wt": wt,
            }
        )
    return ka, kb, in_maps


def kernel(features, edge_src, edge_dst, edge_w, weight):
    global LAST_RESULT
    from concourse.bass_utils import run_bass_kernel_spmd

    ka, kb, in_maps = prepare_inputs(features, edge_src, edge_dst, edge_w, weight)
    key = (ka, kb)
    if key not in _NC_CACHE:
        _NC_CACHE[key] = _build_nc(ka, kb)
    nc = _NC_CACHE[key]

    res = run_bass_kernel_spmd(nc, in_maps, list(range(N_CORES)))
    LAST_RESULT = res
    out = np.concatenate([res.results[c]["out"] for c in range(N_CORES)], axis=0)
    return out.astype(np.float32)


# revision 4
# speedup vs baseline: 1.1550x; 1.1550x over previous
"""GCN layer on 8 Trainium2 NeuronCores — matmul-reduction design (v2).

support = scatter_add(features[src] * w, dst); out = support @ W.T

Per-core sharding: core c owns dst rows [c*6250, (c+1)*6250). Edges are
grouped by dst-tile (128 rows) and dst-group (32 rows within a tile). Each
group's edges occupy whole 128-slot "columns"; a host-built blockW matrix
[128 slots, 32 dsts] (bf16, carrying w) routes each slot's message to its
dst row via the TensorEngine:

    supT_tile[:, 32g:32g+32] += gathered_col[128, 64]^T @ blockW_col[128, 32]

(lhsT = gathered messages -> out is support^T [64, dst], so no transpose is
needed before the final  out = supT^T @ W^T  matmul.)

The gather source is a per-section compact feature table (unique src rows
of ~13 dst-tiles' edges, always < 32767 rows, so int16 gather indices work
with a per-section base). Tables are bf16 padded to 128 cols (gather rows
must be 256B). No dma_scatter_add anywhere (HW loses duplicate-index adds
within a call), and the DVE multiply stage is folded into blockW.

All shapes are static and shared across the 8 cores (SPMD): per-(tile,
group) column counts and per-section table capacities are maxima over
cores; shortfalls are padded with zero-weight slots / zero rows.
"""

import numpy as np

N_NODES = 50000
D = 64
N_CORES = 8
NLOC = N_NODES // N_CORES  # 6250
TILE = 128  # dst rows per tile
GRP = 32  # dst rows per group (matmul M)
NT = (NLOC + TILE - 1) // TILE  # 49 tiles
NGRP = TILE // GRP  # 4 groups per tile
SEC_TILES = 13  # tiles per gather-table section
CHCOLS = 8  # columns (128 slots each) per dma_gather call; 8*128=1024 idxs
# (empirically: dma_gather calls with >1024 indices crash this runtime —
# 1024 passes, 1280 fails — so calls are capped at 1024 indices)

_NC_CACHE = {}
LAST_RESULT = None


def _plan(per_core_edges):
    """Static plan shared by all cores.

    per_core_edges: list of (src, dst_local, w) per core.
    Returns dict with ncols[t][g], col_base[t][g], total_cols, section list,
    ucap per section.
    """
    deg = np.zeros((N_CORES, NT, NGRP), np.int64)
    for c, (src, dstl, w) in enumerate(per_core_edges):
        t = dstl // TILE
        g = (dstl % TILE) // GRP
        np.add.at(deg, (c, t, g), 1)
    ncols = np.maximum(1, -(-deg.max(axis=0) // 128))  # [NT, NGRP]

    col_base = np.zeros((NT, NGRP), np.int64)
    acc = 0
    for t in range(NT):
        for g in range(NGRP):
            col_base[t, g] = acc
            acc += ncols[t, g]
    total_cols = int(acc)

    sections = []  # (tile_lo, tile_hi)
    for lo in range(0, NT, SEC_TILES):
        sections.append((lo, min(lo + SEC_TILES, NT)))
    return dict(
        ncols=ncols, col_base=col_base, total_cols=total_cols, sections=sections
    )


def prepare_inputs(features, edge_src, edge_dst, edge_w, weight):
    import ml_dtypes

    bf16 = ml_dtypes.bfloat16
    features = np.asarray(features, dtype=np.float32)
    src = np.asarray(edge_src).astype(np.int64)
    dst = np.asarray(edge_dst).astype(np.int64)
    w = np.asarray(edge_w, dtype=np.float32)
    weight = np.asarray(weight, dtype=np.float32)

    core = dst // NLOC
    order = np.argsort(core, kind="stable")
    bounds = np.searchsorted(core[order], np.arange(N_CORES + 1))
    per_core = []
    for c in range(N_CORES):
        sl = order[bounds[c] : bounds[c + 1]]
        per_core.append((src[sl], dst[sl] - c * NLOC, w[sl]))

    plan = _plan(per_core)
    ncols, col_base, total_cols = plan["ncols"], plan["col_base"], plan["total_cols"]
    sections = plan["sections"]
    total_slots = total_cols * 128

    # Per-core, per-section unique-src tables
    uniq_per = [[] for _ in range(N_CORES)]  # [core][sec] -> uniq array
    for c in range(N_CORES):
        s_, d_, _ = per_core[c]
        t_ = d_ // TILE
        for lo, hi in sections:
            m = (t_ >= lo) & (t_ < hi)
            uniq_per[c].append(np.unique(s_[m]))
    ucap = [max(len(uniq_per[c][i]) for c in range(N_CORES)) for i in range(len(sections))]
    assert all(u <= 32767 for u in ucap), f"section table too big: {ucap}"
    sec_base = np.concatenate(([0], np.cumsum(ucap)))[: len(sections)]
    tbl_rows = int(sum(ucap))

    featw = features.astype(bf16)
    wt = np.ascontiguousarray(weight.T)  # [d_in, d_out] f32

    in_maps = []
    fill_stats = []
    for c in range(N_CORES):
        s_, d_, w_ = per_core[c]
        t_ = d_ // TILE
        g_ = (d_ % TILE) // GRP
        m_ = (d_ % TILE) % GRP
        # position within (t, g): stable order
        key = t_ * NGRP + g_
        o2 = np.argsort(key, kind="stable")
        s_, d_, w_, t_, g_, m_ = (x[o2] for x in (s_, d_, w_, t_, g_, m_))
        key = key[o2]
        # j = index within its (t,g) run
        starts = np.searchsorted(key, np.arange(NT * NGRP))
        j = np.arange(len(key)) - starts[key]
        col = col_base[t_, g_] + j // 128
        p = j % 128
        slot = col * 128 + p

        # blockW [total_cols, 128, GRP]
        blkw = np.zeros((total_cols, 128, GRP), np.float32)
        blkw[col, p, m_] = w_
        blkw_dev = np.ascontiguousarray(
            blkw.transpose(1, 0, 2).reshape(128, total_cols * GRP).astype(bf16)
        )

        # gather idx per slot (section-local)
        gidx_l = np.zeros(total_slots, np.int64)
        tbl = np.zeros((tbl_rows, 128), bf16)
        for i, (lo, hi) in enumerate(sections):
            uniq = uniq_per[c][i]
            msec = (t_ >= lo) & (t_ < hi)
            gidx_l[slot[msec]] = np.searchsorted(uniq, s_[msec])
            tbl[sec_base[i] : sec_base[i] + len(uniq), :D] = featw[uniq]

        a = gidx_l.astype(np.int16).reshape(total_slots // 16, 16).T
        gidx_dev = np.ascontiguousarray(np.tile(a, (8, 1)))

        in_maps.append({"tbl": tbl, "gidx": gidx_dev, "blkw": blkw_dev, "wt": wt})
        fill_stats.append(len(s_) / total_slots)

    meta = dict(
        ncols=tuple(map(tuple, ncols)),
        ucap=tuple(ucap),
        sec_base=tuple(int(x) for x in sec_base),
        sections=tuple(sections),
        total_cols=total_cols,
        tbl_rows=tbl_rows,
    )
    return meta, in_maps, fill_stats


def _build_nc(meta):
    from contextlib import ExitStack

    import concourse.bass as bass
    import concourse.tile as tile
    from concourse import bacc, mybir

    f32 = mybir.dt.float32
    bf16 = mybir.dt.bfloat16
    i16 = mybir.dt.int16

    ncols = meta["ncols"]
    ucap = meta["ucap"]
    sec_base = meta["sec_base"]
    sections = meta["sections"]
    total_cols = meta["total_cols"]
    tbl_rows = meta["tbl_rows"]
    total_slots = total_cols * 128

    # columns per tile / tile slot bases
    tile_cols = [sum(ncols[t]) for t in range(NT)]
    tile_col_base = np.concatenate(([0], np.cumsum(tile_cols)))

    nc = bacc.Bacc("TRN2", target_bir_lowering=False, debug=False)
    tbl = nc.dram_tensor("tbl", [tbl_rows, 128], bf16, kind="ExternalInput")
    gidx = nc.dram_tensor("gidx", [128, total_slots // 16], i16, kind="ExternalInput")
    blkw = nc.dram_tensor("blkw", [128, total_cols * GRP], bf16, kind="ExternalInput")
    wt = nc.dram_tensor("wt", [D, D], f32, kind="ExternalInput")
    out = nc.dram_tensor("out", [NLOC, D], f32, kind="ExternalOutput")

    # gather chunks: CHCOLS columns at a time, never spanning sections
    chunks = []  # (sec_idx, col_lo, col_hi)
    for i, (lo, hi) in enumerate(sections):
        c_lo, c_hi = int(tile_col_base[lo]), int(tile_col_base[hi])
        for c0 in range(c_lo, c_hi, CHCOLS):
            chunks.append((i, c0, min(c0 + CHCOLS, c_hi)))
    col2chunk = {}
    for k, (_, c0, c1) in enumerate(chunks):
        for col in range(c0, c1):
            col2chunk[col] = (k, col - c0)

    with tile.TileContext(nc) as tc, ExitStack() as ctx:
        cpool = ctx.enter_context(tc.tile_pool(name="const", bufs=1))
        gpool = ctx.enter_context(tc.tile_pool(name="gather", bufs=6))
        fpool = ctx.enter_context(tc.tile_pool(name="fin", bufs=3))
        ppool = ctx.enter_context(tc.tile_pool(name="psum", bufs=4, space="PSUM"))
        opool = ctx.enter_context(tc.tile_pool(name="opsum", bufs=2, space="PSUM"))

        gidx_sb = cpool.tile([128, total_slots // 16], i16)
        nc.sync.dma_start(gidx_sb[:], gidx[:])
        blkw_sb = cpool.tile([128, total_cols * GRP], bf16)
        nc.sync.dma_start(blkw_sb[:], blkw[:])
        wt_sb = cpool.tile([D, D], f32)
        nc.sync.dma_start(wt_sb[:], wt[:])

        def emit_tile(t, gt_handles):
            r = min(TILE, NLOC - t * TILE)
            supT = ppool.tile([D, TILE], f32, tag="supT")
            for g in range(NGRP):
                gbase = int(tile_col_base[t] + sum(ncols[t][:g]))
                ncol = ncols[t][g]
                for j in range(ncol):
                    col_g = gbase + j
                    ck, off = col2chunk[col_g]
                    nc.tensor.matmul(
                        supT[:, g * GRP : (g + 1) * GRP],
                        gt_handles[ck][:, off, 0:D],
                        blkw_sb[:, col_g * GRP : (col_g + 1) * GRP],
                        start=(j == 0),
                        stop=(j == ncol - 1),
                    )
            supT_sb = fpool.tile([D, TILE], f32, tag="supT_sb")
            nc.vector.tensor_copy(supT_sb[:, :], supT[:, :])
            om = opool.tile([TILE, D], f32, tag="om")
            nc.tensor.matmul(
                om[:r, :], supT_sb[:, :r], wt_sb[:, :], start=True, stop=True
            )
            os_ = fpool.tile([TILE, D], f32, tag="os")
            nc.vector.tensor_copy(os_[:r, :], om[:r, :])
            nc.sync.dma_start(out[t * TILE : t * TILE + r, :], os_[:r, :])

        gt_handles = []
        done_tile = 0
        for k, (sec_i, c0, c1) in enumerate(chunks):
            ccols = c1 - c0
            nsl = ccols * 128
            gt = gpool.tile([128, CHCOLS, 128], bf16, tag="gt")
            nc.gpsimd.dma_gather(
                gt[:, :ccols, :],
                tbl[sec_base[sec_i] : sec_base[sec_i] + ucap[sec_i], :],
                gidx_sb[:, c0 * 8 : c1 * 8],
                nsl,
                nsl,
                128,
            )
            gt_handles.append(gt)
            while done_tile < NT and int(tile_col_base[done_tile + 1]) <= c1:
                emit_tile(done_tile, gt_handles)
                done_tile += 1
        assert done_tile == NT

    nc.compile()
    return nc


def kernel(features, edge_src, edge_dst, edge_w, weight):
    global LAST_RESULT
    from concourse.bass_utils import run_bass_kernel_spmd

    meta, in_maps, fill = prepare_inputs(features, edge_src, edge_dst, edge_w, weight)
    key = (meta["ncols"], meta["ucap"])
    if key not in _NC_CACHE:
        _NC_CACHE[key] = _build_nc(meta)
    nc = _NC_CACHE[key]

    res = run_bass_kernel_spmd(nc, in_maps, list(range(N_CORES)))
    LAST_RESULT = res
    out = np.concatenate([res.results[c]["out"] for c in range(N_CORES)], axis=0)
    return out.astype(np.float32)
